# revision 15
# baseline (speedup 1.0000x reference)
"""Trainium2 Bass kernel for sub-center ArcFace (class-parallel over 8 NeuronCores).

Reference math:
  xn = x / ||x||; wn = w / ||w||          (L2 over embed dim, eps=1e-12)
  cos = (xn @ wn.T).reshape(B, C, K).max(-1)           -> logits [B, C]
  phi = cos*cos(m) - sin(theta)*sin(m), guarded; applied at (b, label_b)
  out = (logits, (onehot*phi + (1-onehot)*cos) * 30)

Sharding: class dim split across 8 cores (6250 classes / 18750 weight rows per
core), classic classification-parallel — no collectives. Labels broadcast;
each core applies the margin only to classes it owns, via an indirect-DMA
scatter of the <=1024 corrected logits.

Device-side per chunk of 512 classes: weight-row norms via ACT squares +
ones[128,128] matmul (PSUM accumulate over d broadcasts norms^2 to every
partition for free), rsqrt, scale -> normalized bf16 weights; per batch tile
3x4 bf16 matmuls (x^T stationary) into 3 PSUM banks (one per sub-center),
sub-center max on DVE, x-row scaling folded in by host-normalizing x.
"""

import os
import sys

import numpy as np

for _p in ("/opt/trn_rl_repo", "/root/.axon_site/_ro/trn_rl_repo"):
    if os.path.isdir(_p) and _p not in sys.path:
        sys.path.insert(0, _p)

import ml_dtypes  # noqa: E402

import concourse.bass as bass  # noqa: E402
import concourse.tile as tile  # noqa: E402
from concourse import bacc, mybir  # noqa: E402
from concourse.bass_utils import run_bass_kernel_spmd  # noqa: E402

# Problem constants (hardcoded per task rules)
B = 1024          # batch
D = 512           # embed dim
C = 50000         # num labels
K = 3             # sub-centers
NCORES = 8
CL = C // NCORES  # 6250 classes per core
SCALE = 30.0
MARGIN = 0.3
EPS = 1e-12

COS_M = float(np.cos(MARGIN, dtype=np.float32))
SIN_M = float(np.sin(MARGIN, dtype=np.float32))
TH = float(np.cos(np.pi - MARGIN).astype(np.float32))
MM = float((np.sin(np.pi - MARGIN) * MARGIN).astype(np.float32))

P = 128           # partitions
BT = B // P       # 8 batch tiles
DT = D // P       # 4 contraction chunks
CHUNK = 512       # class chunk (PSUM bank width in fp32)
NCHUNK = (CL + CHUNK - 1) // CHUNK  # 13 (12*512 + 106)
OOB = 1 << 30     # scatter offset sentinel for non-owned rows

F32 = mybir.dt.float32
BF16 = mybir.dt.bfloat16
I32 = mybir.dt.int32
AF = mybir.ActivationFunctionType
OP = mybir.AluOpType

_BF16_NP = ml_dtypes.bfloat16

_NC_CACHE = {}


def _body(tc, wt, xt, lbl, d30in, out0, out1, ctx):
    nc = tc.nc

    res = ctx.enter_context(tc.tile_pool(name="res", bufs=1))
    wpool = ctx.enter_context(tc.tile_pool(name="wpool", bufs=3))
    wnpool = ctx.enter_context(tc.tile_pool(name="wnpool", bufs=3))
    sqpool = ctx.enter_context(tc.tile_pool(name="sqpool", bufs=4))
    invpool = ctx.enter_context(tc.tile_pool(name="invpool", bufs=2))
    epi = ctx.enter_context(tc.tile_pool(name="epi", bufs=5))
    pp = ctx.enter_context(tc.tile_pool(name="pp", bufs=7, space="PSUM"))
    npp = ctx.enter_context(tc.tile_pool(name="npp", bufs=1, space="PSUM"))

    # ---------------- prologue: residents ----------------
    xt_s = res.tile([P, DT, B], BF16, tag="xt_s")
    nc.sync.dma_start(xt_s[:], xt[:])

    lbl_s = res.tile([P, BT], F32, tag="lbl_s")
    nc.sync.dma_start(lbl_s[:], lbl[:])

    iota_s = res.tile([P, CL], F32, tag="iota_s")
    nc.gpsimd.iota(iota_s[:], pattern=[[1, CL]], base=0, channel_multiplier=0,
                   allow_small_or_imprecise_dtypes=True)

    ones_s = res.tile([P, P], BF16, tag="ones_s")
    nc.vector.memset(ones_s[:], 1.0)

    d30 = res.tile([P, BT], F32, tag="d30")     # 30*(phi_guarded - cos) at label
    nc.sync.dma_start(d30[:], d30in[:])

    # ---------------- main loop over class chunks ----------------
    def prepare(ci):
        """DMA chunk ci's weights and produce normalized bf16 columns."""
        c0 = ci * CHUNK
        cw = min(CHUNK, CL - c0)
        wt_c = wpool.tile([P, K * DT, CHUNK], BF16, tag="wt_c",
                          name=f"wt_c{ci}")
        nc.sync.dma_start(wt_c[:, :, :cw], wt[:, :, c0:c0 + cw])
        wn_c = wnpool.tile([P, K * DT, CHUNK], BF16, tag="wn_c",
                           name=f"wn_c{ci}")
        for j in range(K):
            nps = npp.tile([P, CHUNK], F32, tag="nps", name=f"nps{ci}_{j}")
            for d in range(DT):
                wsq = sqpool.tile([P, CHUNK], BF16, tag="wsq",
                                  name=f"wsq{ci}_{j}_{d}")
                nc.scalar.activation(wsq[:, :cw], wt_c[:, j * DT + d, :cw],
                                     AF.Square)
                nc.tensor.matmul(nps[:, :cw], ones_s[:], wsq[:, :cw],
                                 start=(d == 0), stop=(d == DT - 1))
            rec = invpool.tile([P, CHUNK], F32, tag="rec", name=f"rec{ci}_{j}")
            nc.vector.reciprocal(rec[:, :cw], nps[:, :cw])
            invb = invpool.tile([P, CHUNK], BF16, tag="invb",
                                name=f"invb{ci}_{j}")
            nc.scalar.activation(invb[:, :cw], rec[:, :cw], AF.Sqrt)
            for d in range(DT):
                nc.vector.tensor_tensor(wn_c[:, j * DT + d, :cw],
                                        wt_c[:, j * DT + d, :cw],
                                        invb[:, :cw], OP.mult)
        return wn_c

    wn_q = [prepare(0), prepare(1)]
    for ci in range(NCHUNK):
        c0 = ci * CHUNK
        cw = min(CHUNK, CL - c0)
        wn_c = wn_q.pop(0)

        for t in range(BT):
            ps = [pp.tile([P, CHUNK], F32, tag="ps", name=f"ps{jj}")
                  for jj in range(K)]
            for d in range(DT):
                lhs = xt_s[:, d, t * P:(t + 1) * P]
                for j in range(K):
                    nc.tensor.matmul(ps[j][:, :cw], lhs,
                                     wn_c[:, j * DT + d, :cw],
                                     start=(d == 0), stop=(d == DT - 1))
            s0 = epi.tile([P, CHUNK], F32, tag="s0")
            nc.scalar.activation(s0[:, :cw], ps[0][:, :cw], AF.Copy)
            m1 = epi.tile([P, CHUNK], F32, tag="m1")
            nc.vector.tensor_tensor(m1[:, :cw], s0[:, :cw], ps[1][:, :cw],
                                    OP.max)
            o0 = epi.tile([P, CHUNK], F32, tag="o0")
            nc.vector.tensor_tensor(o0[:, :cw], m1[:, :cw], ps[2][:, :cw],
                                    OP.max)
            o1 = epi.tile([P, CHUNK], F32, tag="o1")
            nc.vector.tensor_scalar(o1[:, :cw], o0[:, :cw], SCALE, None,
                                    op0=OP.mult)
            mdl = epi.tile([P, CHUNK], F32, tag="mdl")
            nc.gpsimd.tensor_scalar(mdl[:, :cw], iota_s[:, c0:c0 + cw],
                                    lbl_s[:, t:t + 1], d30[:, t:t + 1],
                                    op0=OP.is_equal, op1=OP.mult)
            nc.gpsimd.tensor_tensor(o1[:, :cw], o1[:, :cw], mdl[:, :cw],
                                    OP.add)
            nc.sync.dma_start(out0[t * P:(t + 1) * P, c0:c0 + cw], o0[:, :cw])
            nc.sync.dma_start(out1[t * P:(t + 1) * P, c0:c0 + cw], o1[:, :cw])

        if ci + 2 < NCHUNK:
            wn_q.append(prepare(ci + 2))


def _build():
    nc = bacc.Bacc("TRN2", debug=False, target_bir_lowering=False)
    wt = nc.dram_tensor("wt", [P, K * DT, CL], BF16, kind="ExternalInput").ap()
    xt = nc.dram_tensor("xt", [P, DT, B], BF16, kind="ExternalInput").ap()
    lbl = nc.dram_tensor("lbl", [P, BT], F32, kind="ExternalInput").ap()
    d30in = nc.dram_tensor("d30in", [P, BT], F32, kind="ExternalInput").ap()
    out0 = nc.dram_tensor("out0", [B, CL], F32, kind="ExternalOutput").ap()
    out1 = nc.dram_tensor("out1", [B, CL], F32, kind="ExternalOutput").ap()

    from contextlib import ExitStack
    with tile.TileContext(nc) as tc:
        with ExitStack() as ctx:
            _body(tc, wt, xt, lbl, d30in, out0, out1, ctx)
    nc.compile()
    return nc


def get_nc():
    if "nc" not in _NC_CACHE:
        _NC_CACHE["nc"] = _build()
    return _NC_CACHE["nc"]


def host_prep(x, labels, weight):
    """Shard + lay out inputs for the 8 cores. Returns list of in_maps."""
    x = np.asarray(x, dtype=np.float32)
    labels = np.asarray(labels).astype(np.int64)
    weight = np.asarray(weight, dtype=np.float32)
    assert x.shape == (B, D) and labels.shape == (B,)
    assert weight.shape == (C * K, D)

    xnorm = x / np.sqrt(np.sum(x * x, axis=1, keepdims=True) + EPS)
    xt_h = np.ascontiguousarray(
        xnorm.T.reshape(DT, P, B).transpose(1, 0, 2)).astype(_BF16_NP)
    w3 = weight.reshape(C, K, D)

    # margin delta for the label cell of each row: 30*(phi_guarded(cos)-cos)
    wlab = w3[labels].astype(np.float32)                         # [B, 3, 512]
    wlab /= np.sqrt(np.sum(wlab * wlab, axis=2, keepdims=True) + EPS)
    cosl = np.max(np.einsum("bd,bkd->bk", xnorm, wlab), axis=1)  # [B]
    sine = np.sqrt(np.clip(1.0 - cosl * cosl, 0.0, 1.0))
    phi = cosl * COS_M - sine * SIN_M
    phi = np.where(cosl > TH, phi, cosl - MM)
    d30_h = np.ascontiguousarray(
        (SCALE * (phi - cosl)).reshape(BT, P).T).astype(np.float32)

    in_maps = []
    for c in range(NCORES):
        c0 = c * CL
        ws = w3[c0:c0 + CL].astype(_BF16_NP)                     # [6250,3,512]
        wt_h = np.ascontiguousarray(
            ws.transpose(2, 1, 0)                                # [512,3,6250]
            .reshape(DT, P, K, CL)
            .transpose(1, 2, 0, 3)                               # [128,3,4,6250]
            .reshape(P, K * DT, CL))
        ll = labels - c0
        ll[(ll < 0) | (ll >= CL)] = -1
        lbl_h = np.ascontiguousarray(
            ll.reshape(BT, P).T).astype(np.float32)              # [128, 8]
        in_maps.append({
            "wt": wt_h, "xt": xt_h, "lbl": lbl_h, "d30in": d30_h,
        })
    return in_maps


def run(in_maps, **kwargs):
    nc = get_nc()
    return run_bass_kernel_spmd(nc, in_maps, core_ids=list(range(NCORES)),
                                **kwargs)


def kernel(x, labels, weight):
    in_maps = host_prep(x, labels, weight)
    res = run(in_maps)
    out0 = np.concatenate([r["out0"] for r in res.results], axis=1)
    out1 = np.concatenate([r["out1"] for r in res.results], axis=1)
    return out0, out1
